# revision 48
# baseline (speedup 1.0000x reference)
"""Trainium2 Bass kernel for nn_Decoder_31370441129997.

GRU decoder: 12 sequential steps of (Linear+ReLU) -> 3x GRU cell -> Linear(2),
with the input-layer representation fed back from the last GRU layer's hidden.

Strategy: data-parallel over batch (4096 -> 8 cores x 512). All weights
resident in SBUF. Activations kept feature-major [H, B] so the recurrence
needs no transposes.

Precision split (rel err 1.23e-2 vs the 2e-2 budget): the r,z gate matmuls
(x-side and h-side) run as fp8-e4m3 DoubleRow matmuls — 256-deep contraction
per pass at 2x the bf16 column rate — while the tanh n-gate, input layer and
output projection stay bf16 (numpy sensitivity sim: fp8 anywhere else blows
the budget; r,z are damped by the sigmoid derivative). r,z weights are
quantized at x16 scale and descaled for free via the ACT scale operand;
activations carry dual copies (bf16 + fp8). ~590us vs the 788us all-bf16
baseline (which sat at its 3552-matmul PE floor); PE busy is >91% at the
new 2469-matmul floor, with DVE at ~82% the secondary constraint — further
fp8 conversion (e.g. layer-1's n h-side, NH8 below) measures NEUTRAL
because those regions go DVE-bound.

Structure (mostly carried from the bf16 baseline, rebalanced for fp8):
 - double-buffered hidden state (even/odd step tiles), no copy-backs.
   h' = z*h + (1-z)*n computed as n + z*(h-n): no (1-z) gate ACT needed.
   Gate tensors bf16 for the 2x DVE rate; t1/t2 stay f32 (PSUM reads).
 - PSUM banks: in=2, hn=2, r=2, z=2. The second "in" bank is what lets
   the next step's input-layer matmuls start the moment h2 data lands
   instead of recycling through the last gate chain's t2 read.
 - boundary fill-in (pend): chunks 0+1's full h-side at every boundary;
   at layer boundaries also the x-side pair-(0,1) DoubleRows and pin's
   first half (they read only chunks 0-1 of the previous layer's output,
   ready well before its chunk-3 chain). pr1/pz1 land on banks freed by
   the previous chunk-3's r/z ACTs — gap fillers by construction.
 - input layer: x8 (fp8) relu on ACT — layer 0's x-side waits on it — and
   the bf16 copy on the DVE via (psum + b) max 0, since the DVE's first
   gate op is ~5us out while the ACT queue is the step-start bottleneck.
 - output projection: two accumulating col-groups (M=2 at cols 0/32) on
   the "in" tag, emitted after the input layer; DVE-only reduction
   (copy + add straight into obuf), deferred so it never blocks hot ACTs.
   At t=11 the projection reads the n / z*(h-n) pieces directly (no hnew
   install), chunk 3 in half-batch pieces.
 - layer-2's fp8 h-copy rides the idle GpSimd (consumed a full step
   later); layers 0/1's stay on DVE (they feed the next layer's x-side).
 - DMA: the five step-0-critical transfers fan out over sync/scalar/
   gpsimd queues; bulk weights follow on sync in consumption order, the
   slack-rich h-side weights as whole-tensor descriptors (fewer end-of-
   program semaphores, ~115ns each per engine queue); output writeback
   overlapped with the last step; short PE warm-up during the DMA window.
"""
import sys

sys.path.insert(0, "/opt/trn_rl_repo")

from contextlib import ExitStack

import numpy as np

import concourse.bass as bass
import concourse.tile as tile
from concourse import bacc, mybir
from concourse.bass_utils import run_bass_kernel_spmd

TPRED = 12
H = 512
L = 3
B = 4096
NCORES = 8
BL = B // NCORES  # 512 batch rows per core
KT = H // 128     # contraction chunks
MT = H // 128     # feature tiles per gate

F32 = mybir.dt.float32
BF16 = mybir.dt.bfloat16
E4 = mybir.dt.float8e4
AF = mybir.ActivationFunctionType
ALU = mybir.AluOpType
DRM = mybir.MatmulPerfMode.DoubleRow
CDT = BF16

SW = 16.0          # fp8 r,z weight scale; descaled via ACT scale=1/SW
SINV = 1.0 / SW
NH8 = False        # fp8 h-side n-gate on layer 1: measured neutral, worse numerics

NBIAS = 66
_CACHE = {}


def _build():
    """Build + compile the per-core Bass program (identical on all 8 cores)."""
    nc = bacc.Bacc("TRN2", target_bir_lowering=False, debug=False)

    rep_d = nc.dram_tensor("rep", [128, KT, BL], CDT, kind="ExternalInput").ap()
    win_d = nc.dram_tensor("win", [128, KT, H], CDT, kind="ExternalInput").ap()
    wxrz_d = nc.dram_tensor("wxrz", [128, L, KT, 2 * H], E4,
                            kind="ExternalInput").ap()
    whrz_d = nc.dram_tensor("whrz", [128, L, KT, 2 * H], E4,
                            kind="ExternalInput").ap()
    wxn_d = nc.dram_tensor("wxn", [128, L, KT, H], CDT,
                           kind="ExternalInput").ap()
    whn_d = nc.dram_tensor("whn", [128, L, KT, H], CDT,
                           kind="ExternalInput").ap()
    whn8_d = nc.dram_tensor("whn8", [128, KT, H], E4,
                            kind="ExternalInput").ap()
    wout_d = nc.dram_tensor("wout", [128, KT, 2], CDT, kind="ExternalInput").ap()
    bias_d = nc.dram_tensor("bias", [128, NBIAS], F32, kind="ExternalInput").ap()
    out_d = nc.dram_tensor("out", [2, TPRED, BL], F32, kind="ExternalOutput").ap()

    with tile.TileContext(nc) as tc, ExitStack() as ctx:
        wpool = ctx.enter_context(tc.tile_pool(name="w", bufs=1))
        state = ctx.enter_context(tc.tile_pool(name="state", bufs=1))
        gates = ctx.enter_context(tc.tile_pool(name="gates", bufs=2))
        psum = ctx.enter_context(tc.tile_pool(name="psum", bufs=2, space="PSUM"))

        # ---- state tiles. h[buf][l] holds all KT feature chunks of layer
        # l's hidden: [128, k, BL]. Step t reads h[t%2], writes h[(t+1)%2].
        # h[0][2] doubles as the step-0 representation input. h8 mirrors h
        # in fp8 for the DoubleRow r,z matmuls (x8 likewise mirrors x).
        h = [[state.tile([128, KT, BL], CDT, tag=f"h{b}_{l}", name=f"h{b}_{l}")
              for l in range(L)] for b in range(2)]
        h8 = [[state.tile([128, KT, BL], E4, tag=f"h8{b}_{l}", name=f"h8{b}_{l}")
               for l in range(L)] for b in range(2)]
        x = state.tile([128, KT, BL], CDT, tag="x", name="x")
        x8 = state.tile([128, KT, BL], E4, tag="x8", name="x8")
        obuf = state.tile([2, TPRED, BL], F32, tag="obuf", name="obuf")

        # ---- weight + input DMAs in consumption order (sync queue). The
        # h-side weights trail everything else: they're first consumed by
        # the t=1 pend fill-in, ~30us in.
        win = wpool.tile([128, KT, H], CDT, tag="win")
        bias = wpool.tile([128, NBIAS], F32, tag="bias")
        wxrz = wpool.tile([128, L, KT, 2 * H], E4, tag="wxrz")
        whrz = wpool.tile([128, L, KT, 2 * H], E4, tag="whrz")
        wxn = wpool.tile([128, L, KT, H], CDT, tag="wxn")
        whn = wpool.tile([128, L, KT, H], CDT, tag="whn")
        whn8 = wpool.tile([128, KT, H], E4, tag="whn8")
        wout = wpool.tile([128, KT, 2], CDT, tag="wout")
        # short PE warm-up off a memset tile, sized to finish as the first
        # input DMAs land: the HAM clock gate reaches 8/8 before the real
        # stream. Drained on the idle GpSimd so no DVE op is blocked.
        wm = state.tile([128, BL], CDT, tag="wm", name="wm")
        nc.vector.memset(wm[:], 0.0)
        pw = psum.tile([128, BL], F32, tag="r", name="pw")
        NWARM = 9
        for i in range(NWARM):
            nc.tensor.matmul(pw[:], lhsT=wm[:, 0:128], rhs=wm[:],
                             start=(i == 0), stop=(i == NWARM - 1))
        nc.scalar.copy(wm[:, 0:1], pw[:, 0:1])

        # the five step-0-critical transfers fan out across idle engine
        # queues so their ~600ns descriptor-issue costs overlap instead of
        # serializing in front of the first matmul; the bulk weight stream
        # stays on sync, in consumption order, behind them.
        nc.sync.dma_start(h[0][2][:, 0:2, :], rep_d[:, 0:2, :])
        nc.scalar.dma_start(win[:, 0:2, :], win_d[:, 0:2, :])
        nc.gpsimd.dma_start(h[0][2][:, 2:4, :], rep_d[:, 2:4, :])
        nc.scalar.dma_start(win[:, 2:4, :], win_d[:, 2:4, :])
        nc.sync.dma_start(bias[:], bias_d[:])
        # x-side weights layer by layer (step-0 critical), rz halves first
        # (~256KB each; the r,z matmuls lead each chunk's issue order).
        for l in range(L):
            nc.sync.dma_start(wxrz[:, l, 0:2], wxrz_d[:, l, 0:2])
            nc.sync.dma_start(wxrz[:, l, 2:4], wxrz_d[:, l, 2:4])
            nc.sync.dma_start(wxn[:, l], wxn_d[:, l])
        nc.sync.dma_start(wout[:], wout_d[:])
        # trailing h-side weights as few large descriptors: each DMA adds a
        # semaphore that every engine queue serially drains (~115ns each) at
        # end-of-program, and these transfers have a full step of slack.
        nc.sync.dma_start(whrz[:], whrz_d[:])
        if NH8:
            nc.sync.dma_start(whn[:, 0], whn_d[:, 0])
            nc.sync.dma_start(whn8[:], whn8_d[:])
            nc.sync.dma_start(whn[:, 2], whn_d[:, 2])
        else:
            nc.sync.dma_start(whn[:], whn_d[:])

        def bcol(c):
            return bias[:, c:c + 1]

        def outproj_mm(t):
            # b_out is added host-side after the gather. Two accumulating
            # col-groups (M=2 at cols 0 and 32). Emitted AFTER the input
            # layer's matmuls on the "in" tag: its bank (px m2's) frees a
            # descriptor after px m2's relu pair, so the projection never
            # blocks the in-order PE queue the way a z/hn slot (freed only
            # by the NEXT step's gate ACTs) would.
            hr2 = h[(t + 1) % 2][2]
            po = psum.tile([128, BL], F32, tag="in", bufs=2, name=f"po_{t}")
            for g in (0, 1):
                for j in (0, 1):
                    k = 2 * g + j
                    nc.tensor.matmul(po[32 * g:32 * g + 2, :],
                                     lhsT=wout[:, k, :], rhs=hr2[:, k, :],
                                     start=(j == 0), stop=(j == 1),
                                     tile_position=(0, 32 * g))
            return po

        def outproj_red(t, po):
            # cross-partition reduction, DVE-only (an ACT copy here would
            # block the queue ahead of layer 0's hot sigmoids): stage one
            # group to SBUF, add the other against it into obuf. The DVE
            # hits these ~po-stop, before its first layer-0 gate op.
            c1 = gates.tile([2, BL], F32, tag="os", name=f"c1_{t}")
            nc.vector.tensor_copy(c1[:], po[32:34, :])
            nc.vector.tensor_add(obuf[:, t, :], po[0:2, :], c1[:])

        def hmm_rz(pt, t, l, m, gate):
            """h-side r/z matmuls: fp8 DoubleRow, 256-contraction per pass.
            Keeps the PSUM group open for the x-side accumulation."""
            h8p = h8[t % 2][l]
            lo = gate * H + m * 128
            for kk in range(0, KT, 2):
                nc.tensor.matmul(pt[:], lhsT=whrz[:, l, kk:kk + 2, lo:lo + 128],
                                 rhs=h8p[:, kk:kk + 2, :], start=(kk == 0),
                                 stop=False, perf_mode=DRM)

        def hmm_n(pt, t, l, m):
            """h-side n matmul group; closes its PSUM group. Layer 1 runs
            fp8 DoubleRow off the existing h8 copy (numerics sim: 1.6e-2;
            l2 or any x-side n in fp8 blows the 2e-2 budget). Its weights
            are quantized UNSCALED so ph needs no descale in the STT."""
            lo = m * 128
            if l == 1 and NH8:
                h8p = h8[t % 2][l]
                for kk in range(0, KT, 2):
                    nc.tensor.matmul(pt[:], lhsT=whn8[:, kk:kk + 2, lo:lo + 128],
                                     rhs=h8p[:, kk:kk + 2, :], start=(kk == 0),
                                     stop=(kk == KT - 2), perf_mode=DRM)
                return
            hp = h[t % 2][l]
            for k in range(KT):
                nc.tensor.matmul(pt[:], lhsT=whn[:, l, k, lo:lo + 128],
                                 rhs=hp[:, k, :], start=(k == 0),
                                 stop=(k == KT - 1))

        def pend_block(t, l, x8in=None, xin=None):
            """Boundary fill-in: chunks 0+1's full h-side (n, r, z). 16
            matmuls that depend only on h(t-1), issued before anything
            that waits on the previous block's gate chain. pr1/pz1 land on
            the banks freed by the previous chunk-3's r/z ACTs, ~1us into
            the boundary — gap fillers by construction.

            At LAYER boundaries (x8in given = the current layer's h8/h
            output) the fill also takes the x-side pair-(0,1) DoubleRows
            and pin's first half: those read only chunks 0-1 of the
            previous layer's output, complete well before its chunk-3
            chain — the thing the boundary is waiting on."""
            ph0 = psum.tile([128, BL], F32, tag="hn", bufs=2,
                            name=f"ph_{t}_{l}_0")
            hmm_n(ph0, t, l, 0)
            pr0 = psum.tile([128, BL], F32, tag="r", name=f"pr_{t}_{l}_0")
            hmm_rz(pr0, t, l, 0, 0)
            pz0 = psum.tile([128, BL], F32, tag="z", name=f"pz_{t}_{l}_0")
            hmm_rz(pz0, t, l, 0, 1)
            pin0 = None
            if x8in is not None:
                nc.tensor.matmul(pr0[:], lhsT=wxrz[:, l, 0:2, 0:128],
                                 rhs=x8in[:, 0:2, :], start=False,
                                 stop=False, perf_mode=DRM)
                nc.tensor.matmul(pz0[:], lhsT=wxrz[:, l, 0:2, H:H + 128],
                                 rhs=x8in[:, 0:2, :], start=False,
                                 stop=False, perf_mode=DRM)
                # pin chunk0's first half; its "in" bank (the previous
                # layer's chunk-2 pin) freed a full chunk before the
                # boundary. The second half would land on the chunk-3 bank
                # — the late one — so it stays in-loop.
                pin0 = psum.tile([128, BL], F32, tag="in", bufs=2,
                                 name=f"pin_{t}_{l}_0")
                for k in (0, 1):
                    nc.tensor.matmul(pin0[:], lhsT=wxn[:, l, k, 0:128],
                                     rhs=xin[:, k, :], start=(k == 0),
                                     stop=False)
            ph1 = psum.tile([128, BL], F32, tag="hn", bufs=2,
                            name=f"ph_{t}_{l}_1")
            hmm_n(ph1, t, l, 1)
            pr1 = psum.tile([128, BL], F32, tag="r", name=f"pr_{t}_{l}_1")
            hmm_rz(pr1, t, l, 1, 0)
            pz1 = psum.tile([128, BL], F32, tag="z", name=f"pz_{t}_{l}_1")
            hmm_rz(pz1, t, l, 1, 1)
            if x8in is not None:
                nc.tensor.matmul(pr1[:], lhsT=wxrz[:, l, 0:2, 128:256],
                                 rhs=x8in[:, 0:2, :], start=False,
                                 stop=False, perf_mode=DRM)
                nc.tensor.matmul(pz1[:], lhsT=wxrz[:, l, 0:2, H + 128:H + 256],
                                 rhs=x8in[:, 0:2, :], start=False,
                                 stop=False, perf_mode=DRM)
            return ph0, pr0, pz0, ph1, pr1, pz1, pin0

        pend = None
        e1f, e2f = [], []
        for t in range(TPRED):
            hwr = h[(t + 1) % 2]
            h8w = h8[(t + 1) % 2]
            # input layer: x = relu(W_in @ h2 + b_in), on ACT so the px
            # PSUM bank recycling doesn't wait on the DVE gate chain. The
            # fp8 copy (layer-0 r,z input) is written first: it's on the
            # r-gate critical path.
            hr2 = h[t % 2][2]
            for m in range(MT):
                px = psum.tile([128, BL], F32, tag="in", bufs=2, name=f"px{m}")
                for k in range(KT):
                    nc.tensor.matmul(px[:],
                                     lhsT=win[:, k, m * 128:(m + 1) * 128],
                                     rhs=hr2[:, k, :],
                                     start=(k == 0), stop=(k == KT - 1))
                nc.scalar.activation(x8[:, m, :], px[:], AF.Relu,
                                     bias=bcol(48 + m))
                # bf16 copy on the DVE (reads PSUM + bias col, then max 0):
                # the ACT queue at step start must reach x8 m3 fast — the
                # whole of layer 0's x-side waits on it — while the DVE's
                # first gate op is ~5us out.
                nc.vector.tensor_scalar(x[:, m, :], px[:], bcol(48 + m),
                                        0.0, op0=ALU.add, op1=ALU.max)
            if t > 0:
                po_prev = outproj_mm(t - 1)
                outproj_red(t - 1, po_prev)
            if t == TPRED - 1:
                # overlap the bulk of the output writeback with the last
                # step (after outproj_red(10) has written obuf[:, 10])
                nc.sync.dma_start(out_d[:, :TPRED - 1, :], obuf[:, :TPRED - 1, :])
            for l in range(L):
                xin = x if l == 0 else hwr[l - 1]
                x8in = x8 if l == 0 else h8w[l - 1]
                hro = h[t % 2][l]
                tail_prev = None
                for m in range(MT):
                    lo = m * 128
                    hi = lo + 128
                    ph = pr = pz = pin = None
                    pend_x = False
                    if t > 0:
                        if m == 0:
                            ph, pr, pz = pend[0], pend[1], pend[2]
                            pend_x = pend[6] is not None
                            pin = pend[6]
                        elif m == 1:
                            ph, pr, pz = pend[3], pend[4], pend[5]
                            pend_x = pend[6] is not None
                        else:
                            ph = psum.tile([128, BL], F32, tag="hn", bufs=2,
                                           name=f"ph_{t}_{l}_{m}")
                            hmm_n(ph, t, l, m)
                        if pr is None:
                            pr = psum.tile([128, BL], F32, tag="r",
                                           name=f"pr_{t}_{l}_{m}")
                            hmm_rz(pr, t, l, m, 0)
                            pz = psum.tile([128, BL], F32, tag="z",
                                           name=f"pz_{t}_{l}_{m}")
                            hmm_rz(pz, t, l, m, 1)
                    else:
                        pr = psum.tile([128, BL], F32, tag="r",
                                       name=f"pr_{t}_{l}_{m}")
                        pz = psum.tile([128, BL], F32, tag="z",
                                       name=f"pz_{t}_{l}_{m}")
                    # x-side r,z: fp8 DoubleRow; closes the groups. Pair
                    # (0,1) for both gates ahead of pair (2,3): the pair
                    # only needs input chunks 0-1, giving the producer of
                    # chunk 3 (the previous layer's last gate chain) an
                    # extra matmul of slack (pend_x chunks had pair (0,1)
                    # pre-issued at the boundary). For layer 0's first
                    # chunks the input is the freshly relu'd x/x8, whose
                    # chunk-3 ACTs trail the PE — interleave pin's first
                    # half between the pairs so the PE never waits on the
                    # x8 m3 write.
                    if pin is None:
                        pin = psum.tile([128, BL], F32, tag="in", bufs=2,
                                        name=f"pin_{t}_{l}_{m}")
                        pin_ks = range(KT)
                    else:
                        pin_ks = (2, 3)
                    interleave = (l == 0 and m < 2)
                    kks = (2,) if (pend_x and m < 2) else (0, 2)
                    for kk in kks:
                        nc.tensor.matmul(pr[:],
                                         lhsT=wxrz[:, l, kk:kk + 2, lo:hi],
                                         rhs=x8in[:, kk:kk + 2, :],
                                         start=(t == 0 and kk == 0),
                                         stop=(kk == KT - 2), perf_mode=DRM)
                        nc.tensor.matmul(pz[:],
                                         lhsT=wxrz[:, l, kk:kk + 2, H + lo:H + hi],
                                         rhs=x8in[:, kk:kk + 2, :],
                                         start=(t == 0 and kk == 0),
                                         stop=(kk == KT - 2), perf_mode=DRM)
                        if interleave:
                            for k in (kk, kk + 1):
                                nc.tensor.matmul(pin[:],
                                                 lhsT=wxn[:, l, k, lo:hi],
                                                 rhs=xin[:, k, :],
                                                 start=(k == 0),
                                                 stop=(k == KT - 1))
                    if not interleave:
                        for k in pin_ks:
                            nc.tensor.matmul(pin[:],
                                             lhsT=wxn[:, l, k, lo:hi],
                                             rhs=xin[:, k, :], start=(k == 0),
                                             stop=(k == KT - 1))
                    if m == MT - 1:
                        # next block's boundary fill-in goes into the PE
                        # queue before the ops that wait on this chunk's
                        # gate chain. Only issued when the target step will
                        # actually consume it (t=0 has no h-side).
                        if l < L - 1:
                            if t > 0:
                                pend = pend_block(t, l + 1, h8w[l], hwr[l])
                        elif t < TPRED - 1:
                            pend = pend_block(t + 1, 0)
                            # dependency-free weight loads on the static
                            # warm-up tile: they execute inside the step-
                            # boundary stall and keep the PE p-state clock
                            # from dropping, so the first real matmuls of
                            # the next step run at full rate.
                            for _ in range(6):
                                nc.tensor.ldweights(wm[:, 0:128])

                    # gates for this feature chunk, in bf16 (2x DVE rate).
                    # h' = z*h + (1-z)*n computed as n + z*(h - n) — no
                    # (1-z) gate needed. The 1/16 ACT scale descales the
                    # x16-quantized fp8 r,z weights. For LAYER 2 ONLY the
                    # tanh-onward tail is deferred into the next chunk's
                    # section: every consumer of h2/h8[2] (px, outproj,
                    # next step's l2 h-side) needs chunk 3 last anyway, so
                    # the delay is free — and the in-order ACT queue then
                    # reaches chunk 3's r/z (which free the step-boundary
                    # pend's PSUM banks) without waiting on chunk 2's
                    # t2-dependent tanh. For layers 0/1 the tails stay in
                    # place: their h8 feeds the NEXT layer's x-side, where
                    # a one-chunk delay measurably stalls the PE.
                    final = (t == TPRED - 1 and l == L - 1)
                    defer = False
                    r = gates.tile([128, BL], CDT, tag="r", name=f"r{m}")
                    nc.scalar.activation(r[:], pr[:], AF.Sigmoid,
                                         bias=bcol(l * 16 + m), scale=SINV)
                    z = gates.tile([128, BL], CDT, tag="z", name=f"z{m}")
                    nc.scalar.activation(z[:], pz[:], AF.Sigmoid,
                                         bias=bcol(l * 16 + 4 + m), scale=SINV)
                    if final:
                        # final layer-2: h2(11) is only ever consumed by
                        # the output projection, so keep the n / z*(h-n)
                        # pieces and matmul them directly — no hnew install.
                        nt = gates.tile([128, BL], CDT, tag=f"e1f{m}",
                                        bufs=1, name=f"e1f{m}")
                        mzt = gates.tile([128, BL], CDT, tag=f"e2f{m}",
                                         bufs=1, name=f"e2f{m}")
                        e1f.append(nt)
                        e2f.append(mzt)
                    else:
                        nt = gates.tile([128, BL], CDT, tag="n", bufs=2,
                                        name=f"n{m}")
                        mzt = gates.tile([128, BL], CDT, tag="mz", bufs=2,
                                         name=f"mz{m}")
                    if final and m == MT - 1:
                        # last chunk of the last step: flush chunk 2's
                        # tail, then in-place half-batch pieces so the
                        # closing outproj's last dependency resolves early.
                        if tail_prev is not None:
                            tail_prev()
                            tail_prev = None
                        for sl in (slice(0, BL // 2), slice(BL // 2, BL)):
                            t1 = gates.tile([128, BL // 2], F32, tag="t1h",
                                            name=f"t1h_{sl.start}")
                            nc.vector.scalar_tensor_tensor(
                                t1[:], ph[:, sl], bcol(l * 16 + 8 + m),
                                r[:, sl], op0=ALU.add, op1=ALU.mult)
                            t2 = gates.tile([128, BL // 2], F32, tag="t1h",
                                            name=f"t2h_{sl.start}")
                            nc.vector.tensor_add(t2[:], t1[:], pin[:, sl])
                            nc.scalar.activation(nt[:, sl], t2[:], AF.Tanh,
                                                 bias=bcol(l * 16 + 12 + m))
                            d = gates.tile([128, BL // 2], CDT, tag="dh",
                                           bufs=1, name=f"dh{sl.start}")
                            nc.vector.tensor_sub(d[:], hro[:, m, sl],
                                                 nt[:, sl])
                            nc.vector.tensor_mul(mzt[:, sl], z[:, sl], d[:])
                        continue
                    # stage 1: the PSUM consumers (t1 reads ph, t2 reads pin)
                    t1 = gates.tile([128, BL], F32, tag="t1",
                                    name=f"t1_{m}")
                    if t > 0:
                        # t1 = (hn_psum + b_hh_n) * r
                        nc.vector.scalar_tensor_tensor(
                            t1[:], ph[:], bcol(l * 16 + 8 + m), r[:],
                            op0=ALU.add, op1=ALU.mult)
                    else:
                        nc.vector.tensor_scalar(t1[:], r[:],
                                                bcol(l * 16 + 8 + m),
                                                None, op0=ALU.mult)
                    t2 = gates.tile([128, BL], F32, tag="t1",
                                    name=f"t2_{m}")
                    nc.vector.tensor_add(t2[:], t1[:], pin[:])
                    if tail_prev is not None:
                        tail_prev()
                        tail_prev = None

                    def _tail(t=t, l=l, m=m, nt=nt, mzt=mzt, z=z, t2=t2,
                              hro=hro, hwr=hwr, h8w=h8w, final=final):
                        nc.scalar.activation(nt[:], t2[:], AF.Tanh,
                                             bias=bcol(l * 16 + 12 + m))
                        if t > 0:
                            d = gates.tile([128, BL], CDT, tag="d", bufs=1,
                                           name=f"d{m}")
                            nc.vector.tensor_sub(d[:], hro[:, m, :], nt[:])
                            nc.vector.tensor_mul(mzt[:], z[:], d[:])
                        else:
                            # h' = (1 - z) * n = n - z*n
                            nc.vector.tensor_mul(mzt[:], z[:], nt[:])
                        if final:
                            return
                        op = nc.vector.tensor_add if t > 0 else \
                            nc.vector.tensor_sub
                        gop = nc.gpsimd.tensor_add if t > 0 else \
                            nc.gpsimd.tensor_sub
                        if l == L - 1:
                            # h2 feeds the outproj + next input layer first.
                            # Its fp8 copy isn't consumed until the next
                            # step's layer-2 h-side, so the slow-but-idle
                            # GpSimd takes it off the DVE queue.
                            op(hwr[l][:, m, :], nt[:], mzt[:])
                            gop(h8w[l][:, m, :], nt[:], mzt[:])
                        else:
                            # h8 feeds the next layer's x-side DR matmuls
                            op(h8w[l][:, m, :], nt[:], mzt[:])
                            op(hwr[l][:, m, :], nt[:], mzt[:])
                    if defer:
                        tail_prev = _tail
                    else:
                        _tail()
                if tail_prev is not None:
                    tail_prev()
                    tail_prev = None
        # final outproj straight from the e1/e2 pieces, in two batch halves
        # so the first half's copy+DMA pipeline behind the second half's
        # matmuls; the very last matmul depends only on e2 chunk 3's second
        # half.
        for c in (0, BL // 2):
            sl = slice(c, c + BL // 2)
            poh = psum.tile([2, BL // 2], F32, tag="z", name=f"po_last{c}")
            for k in range(KT):
                nc.tensor.matmul(poh[:], lhsT=wout[:, k, :],
                                 rhs=e1f[k][:, sl], start=(k == 0), stop=False)
            for k in range(KT):
                nc.tensor.matmul(poh[:], lhsT=wout[:, k, :],
                                 rhs=e2f[k][:, sl], start=False,
                                 stop=(k == KT - 1))
            if c == 0:
                nc.scalar.copy(obuf[:, TPRED - 1, sl], poh[:])
                nc.sync.dma_start(out_d[:, TPRED - 1, sl],
                                  obuf[:, TPRED - 1, sl])
            else:
                nc.vector.tensor_scalar(obuf[:, TPRED - 1, sl], poh[:], 0.0,
                                        None, op0=ALU.add)
                nc.scalar.dma_start(out_d[:, TPRED - 1, sl],
                                    obuf[:, TPRED - 1, sl])

    nc.compile()
    return nc


def _to_dev(x):
    import ml_dtypes
    return np.ascontiguousarray(x, dtype=np.float32).astype(ml_dtypes.bfloat16)


def _to_fp8(x):
    import ml_dtypes
    y = np.clip(np.ascontiguousarray(x, dtype=np.float32) * SW, -240.0, 240.0)
    return y.astype(ml_dtypes.float8_e4m3)


def _prep_inputs(representation, W_in, b_in, W_ih, W_hh, b_ih, b_hh, W_out, b_out):
    rep_T = np.ascontiguousarray(representation.reshape(B, H).T)  # [H, B]
    # [H, X] -> [128, KT, X] partition-major tiling of the contraction dim
    win = _to_dev(W_in.T.reshape(KT, 128, H).transpose(1, 0, 2))
    wxrz = _to_fp8(np.transpose(W_ih[:, :2 * H], (0, 2, 1))
                   .reshape(L, KT, 128, 2 * H).transpose(2, 0, 1, 3))
    whrz = _to_fp8(np.transpose(W_hh[:, :2 * H], (0, 2, 1))
                   .reshape(L, KT, 128, 2 * H).transpose(2, 0, 1, 3))
    wxn = _to_dev(np.transpose(W_ih[:, 2 * H:], (0, 2, 1))
                  .reshape(L, KT, 128, H).transpose(2, 0, 1, 3))
    whn = _to_dev(np.transpose(W_hh[:, 2 * H:], (0, 2, 1))
                  .reshape(L, KT, 128, H).transpose(2, 0, 1, 3))
    import ml_dtypes
    whn8 = np.clip(np.ascontiguousarray(
        W_hh[1, 2 * H:].T.reshape(KT, 128, H).transpose(1, 0, 2),
        dtype=np.float32), -240.0, 240.0).astype(ml_dtypes.float8_e4m3)
    wout = _to_dev(W_out.T.reshape(KT, 128, 2).transpose(1, 0, 2))

    bias = np.zeros((128, NBIAS), dtype=np.float32)
    brz = (b_ih[:, :2 * H] + b_hh[:, :2 * H]).astype(np.float32)  # [L, 2H]
    for l in range(L):
        for g in range(2):
            for m in range(MT):
                bias[:, l * 16 + g * 4 + m] = brz[l, g * H + m * 128:
                                                  g * H + (m + 1) * 128]
        for m in range(MT):
            bias[:, l * 16 + 8 + m] = b_hh[l, 2 * H + m * 128:2 * H + (m + 1) * 128]
            bias[:, l * 16 + 12 + m] = b_ih[l, 2 * H + m * 128:2 * H + (m + 1) * 128]
            # negated z bias for q = sigmoid(-(pz/16 + b_z))
            bias[:, 53 + l * 4 + m] = -brz[l, H + m * 128:H + (m + 1) * 128]
    for m in range(MT):
        bias[:, 48 + m] = b_in[m * 128:(m + 1) * 128]

    shared = {"win": win, "wxrz": wxrz, "whrz": whrz, "wxn": wxn, "whn": whn,
              "whn8": whn8, "wout": wout, "bias": bias}
    in_maps = []
    for c in range(NCORES):
        m = dict(shared)
        rep_c = rep_T[:, c * BL:(c + 1) * BL]          # [H, BL]
        m["rep"] = _to_dev(rep_c.reshape(KT, 128, BL).transpose(1, 0, 2))
        in_maps.append(m)
    return in_maps


def _run(inputs, trace=False):
    if "nc" not in _CACHE:
        _CACHE["nc"] = _build()
    nc = _CACHE["nc"]
    in_maps = _prep_inputs(
        inputs["representation"], inputs["W_in"], inputs["b_in"],
        inputs["W_ih"], inputs["W_hh"], inputs["b_ih"], inputs["b_hh"],
        inputs["W_out"], inputs["b_out"])
    res = run_bass_kernel_spmd(nc, in_maps, core_ids=list(range(NCORES)),
                               trace=trace)
    # per-core out: [2, TPRED, BL] -> full [B, TPRED, 2]
    full = np.empty((B, TPRED, 2), dtype=np.float32)
    for c in range(NCORES):
        o = res.results[c]["out"]                      # [2, 12, BL]
        full[c * BL:(c + 1) * BL] = np.transpose(o, (2, 1, 0))
    full += inputs["b_out"].astype(np.float32)[None, None, :]
    return full, res


def kernel(**inputs) -> np.ndarray:
    out, _ = _run(inputs, trace=False)
    return out


def _setup_tracing():
    """Register the NTFF profile hook shim (test harness only)."""
    import types

    import trn_agent_boot.trn_boot as tb

    mod = types.ModuleType("antenv.axon_hooks")
    hook = [tb._ntff_profile_via_ctypes("/opt/axon/libaxon_pjrt.so")]
    mod.get_axon_ntff_profile_hook = lambda: hook[0]
    mod.set_axon_ntff_profile_hook = lambda h: hook.__setitem__(0, h)
    sys.modules["antenv.axon_hooks"] = mod
    import antenv
    antenv.axon_hooks = mod

    from concourse import bass_utils
    bass_utils.upload_artifacts = lambda tmpdir: str(tmpdir)


# revision 49
# speedup vs baseline: 1.0094x; 1.0094x over previous
"""Trainium2 Bass kernel for nn_Decoder_31370441129997.

GRU decoder: 12 sequential steps of (Linear+ReLU) -> 3x GRU cell -> Linear(2),
with the input-layer representation fed back from the last GRU layer's hidden.

Strategy: data-parallel over batch (4096 -> 8 cores x 512). All weights
resident in SBUF. Activations kept feature-major [H, B] so the recurrence
needs no transposes.

Precision split (rel err 1.23e-2 vs the 2e-2 budget): the r,z gate matmuls
(x-side and h-side) run as fp8-e4m3 DoubleRow matmuls — 256-deep contraction
per pass at 2x the bf16 column rate — while the tanh n-gate, input layer and
output projection stay bf16 (numpy sensitivity sim: fp8 anywhere else blows
the budget; r,z are damped by the sigmoid derivative). r,z weights are
quantized at x16 scale and descaled for free via the ACT scale operand;
activations carry dual copies (bf16 + fp8). ~590us vs the 788us all-bf16
baseline (which sat at its 3552-matmul PE floor); PE busy is >91% at the
new 2469-matmul floor, with DVE at ~82% the secondary constraint — further
fp8 conversion (e.g. layer-1's n h-side, NH8 below) measures NEUTRAL
because those regions go DVE-bound.

Structure (mostly carried from the bf16 baseline, rebalanced for fp8):
 - double-buffered hidden state (even/odd step tiles), no copy-backs.
   h' = z*h + (1-z)*n computed as n + z*(h-n): no (1-z) gate ACT needed.
   Gate tensors bf16 for the 2x DVE rate; t1/t2 stay f32 (PSUM reads).
 - PSUM banks: in=2, hn=2, r=2, z=2. The second "in" bank is what lets
   the next step's input-layer matmuls start the moment h2 data lands
   instead of recycling through the last gate chain's t2 read.
 - boundary fill-in (pend): chunks 0+1's full h-side at every boundary;
   at layer boundaries also the x-side pair-(0,1) DoubleRows and pin's
   first half (they read only chunks 0-1 of the previous layer's output,
   ready well before its chunk-3 chain). pr1/pz1 land on banks freed by
   the previous chunk-3's r/z ACTs — gap fillers by construction.
 - input layer: x8 (fp8) relu on ACT — layer 0's x-side waits on it — and
   the bf16 copy on the DVE via (psum + b) max 0, since the DVE's first
   gate op is ~5us out while the ACT queue is the step-start bottleneck.
 - output projection: two accumulating col-groups (M=2 at cols 0/32) on
   the "in" tag, emitted after the input layer; DVE-only reduction
   (copy + add straight into obuf), deferred so it never blocks hot ACTs.
   At t=11 the projection reads the n / z*(h-n) pieces directly (no hnew
   install), chunk 3 in half-batch pieces.
 - layer-2's fp8 h-copy rides the idle GpSimd (consumed a full step
   later); layers 0/1's stay on DVE (they feed the next layer's x-side).
 - DMA: the five step-0-critical transfers fan out over sync/scalar/
   gpsimd queues; bulk weights follow on sync in consumption order, the
   slack-rich h-side weights as whole-tensor descriptors (fewer end-of-
   program semaphores, ~115ns each per engine queue); output writeback
   overlapped with the last step; short PE warm-up during the DMA window.
"""
import sys

sys.path.insert(0, "/opt/trn_rl_repo")

from contextlib import ExitStack

import numpy as np

import concourse.bass as bass
import concourse.tile as tile
from concourse import bacc, mybir
from concourse.bass_utils import run_bass_kernel_spmd

TPRED = 12
H = 512
L = 3
B = 4096
NCORES = 8
BL = B // NCORES  # 512 batch rows per core
KT = H // 128     # contraction chunks
MT = H // 128     # feature tiles per gate

F32 = mybir.dt.float32
BF16 = mybir.dt.bfloat16
E4 = mybir.dt.float8e4
AF = mybir.ActivationFunctionType
ALU = mybir.AluOpType
DRM = mybir.MatmulPerfMode.DoubleRow
CDT = BF16

SW = 16.0          # fp8 r,z weight scale; descaled via ACT scale=1/SW
SINV = 1.0 / SW
NH8 = False        # fp8 h-side n-gate on layer 1: measured neutral, worse numerics

NBIAS = 66
_CACHE = {}


def _build():
    """Build + compile the per-core Bass program (identical on all 8 cores)."""
    nc = bacc.Bacc("TRN2", target_bir_lowering=False, debug=False)

    rep_d = nc.dram_tensor("rep", [128, KT, BL], CDT, kind="ExternalInput").ap()
    win_d = nc.dram_tensor("win", [128, KT, H], CDT, kind="ExternalInput").ap()
    wxrz_d = nc.dram_tensor("wxrz", [128, L, KT, 2 * H], E4,
                            kind="ExternalInput").ap()
    whrz_d = nc.dram_tensor("whrz", [128, L, KT, 2 * H], E4,
                            kind="ExternalInput").ap()
    wxn_d = nc.dram_tensor("wxn", [128, L, KT, H], CDT,
                           kind="ExternalInput").ap()
    whn_d = nc.dram_tensor("whn", [128, L, KT, H], CDT,
                           kind="ExternalInput").ap()
    whn8_d = nc.dram_tensor("whn8", [128, KT, H], E4,
                            kind="ExternalInput").ap()
    wout_d = nc.dram_tensor("wout", [128, KT, 2], CDT, kind="ExternalInput").ap()
    bias_d = nc.dram_tensor("bias", [128, NBIAS], F32, kind="ExternalInput").ap()
    out_d = nc.dram_tensor("out", [2, TPRED, BL], F32, kind="ExternalOutput").ap()

    with tile.TileContext(nc) as tc, ExitStack() as ctx:
        wpool = ctx.enter_context(tc.tile_pool(name="w", bufs=1))
        state = ctx.enter_context(tc.tile_pool(name="state", bufs=1))
        gates = ctx.enter_context(tc.tile_pool(name="gates", bufs=2))
        psum = ctx.enter_context(tc.tile_pool(name="psum", bufs=2, space="PSUM"))

        # ---- state tiles. h[buf][l] holds all KT feature chunks of layer
        # l's hidden: [128, k, BL]. Step t reads h[t%2], writes h[(t+1)%2].
        # h[0][2] doubles as the step-0 representation input. h8 mirrors h
        # in fp8 for the DoubleRow r,z matmuls (x8 likewise mirrors x).
        h = [[state.tile([128, KT, BL], CDT, tag=f"h{b}_{l}", name=f"h{b}_{l}")
              for l in range(L)] for b in range(2)]
        h8 = [[state.tile([128, KT, BL], E4, tag=f"h8{b}_{l}", name=f"h8{b}_{l}")
               for l in range(L)] for b in range(2)]
        x = state.tile([128, KT, BL], CDT, tag="x", name="x")
        x8 = state.tile([128, KT, BL], E4, tag="x8", name="x8")
        obuf = state.tile([2, TPRED, BL], F32, tag="obuf", name="obuf")

        # ---- weight + input DMAs in consumption order (sync queue). The
        # h-side weights trail everything else: they're first consumed by
        # the t=1 pend fill-in, ~30us in.
        win = wpool.tile([128, KT, H], CDT, tag="win")
        bias = wpool.tile([128, NBIAS], F32, tag="bias")
        wxrz = wpool.tile([128, L, KT, 2 * H], E4, tag="wxrz")
        whrz = wpool.tile([128, L, KT, 2 * H], E4, tag="whrz")
        wxn = wpool.tile([128, L, KT, H], CDT, tag="wxn")
        whn = wpool.tile([128, L, KT, H], CDT, tag="whn")
        whn8 = wpool.tile([128, KT, H], E4, tag="whn8")
        wout = wpool.tile([128, KT, 2], CDT, tag="wout")
        # short PE warm-up off a memset tile, sized to finish as the first
        # input DMAs land: the HAM clock gate reaches 8/8 before the real
        # stream. Drained on the idle GpSimd so no DVE op is blocked.
        wm = state.tile([128, BL], CDT, tag="wm", name="wm")
        nc.vector.memset(wm[:], 0.0)
        pw = psum.tile([128, BL], F32, tag="r", name="pw")
        NWARM = 9
        for i in range(NWARM):
            nc.tensor.matmul(pw[:], lhsT=wm[:, 0:128], rhs=wm[:],
                             start=(i == 0), stop=(i == NWARM - 1))
        nc.scalar.copy(wm[:, 0:1], pw[:, 0:1])

        # the five step-0-critical transfers fan out across idle engine
        # queues so their ~600ns descriptor-issue costs overlap instead of
        # serializing in front of the first matmul; the bulk weight stream
        # stays on sync, in consumption order, behind them.
        nc.sync.dma_start(h[0][2][:, 0:2, :], rep_d[:, 0:2, :])
        nc.scalar.dma_start(win[:, 0:2, :], win_d[:, 0:2, :])
        nc.gpsimd.dma_start(h[0][2][:, 2:4, :], rep_d[:, 2:4, :])
        nc.scalar.dma_start(win[:, 2:4, :], win_d[:, 2:4, :])
        nc.sync.dma_start(bias[:], bias_d[:])
        # x-side weights layer by layer (step-0 critical), rz halves first
        # (~256KB each; the r,z matmuls lead each chunk's issue order).
        for l in range(L):
            nc.sync.dma_start(wxrz[:, l, 0:2], wxrz_d[:, l, 0:2])
            nc.sync.dma_start(wxrz[:, l, 2:4], wxrz_d[:, l, 2:4])
            nc.sync.dma_start(wxn[:, l], wxn_d[:, l])
        nc.sync.dma_start(wout[:], wout_d[:])
        # trailing h-side weights as few large descriptors: each DMA adds a
        # semaphore that every engine queue serially drains (~115ns each) at
        # end-of-program, and these transfers have a full step of slack.
        nc.sync.dma_start(whrz[:], whrz_d[:])
        if NH8:
            nc.sync.dma_start(whn[:, 0], whn_d[:, 0])
            nc.sync.dma_start(whn8[:], whn8_d[:])
            nc.sync.dma_start(whn[:, 2], whn_d[:, 2])
        else:
            nc.sync.dma_start(whn[:], whn_d[:])

        def bcol(c):
            return bias[:, c:c + 1]

        def outproj_mm(t):
            # b_out is added host-side after the gather. Two accumulating
            # col-groups (M=2 at cols 0 and 32). Emitted AFTER the input
            # layer's matmuls on the "in" tag: its bank (px m2's) frees a
            # descriptor after px m2's relu pair, so the projection never
            # blocks the in-order PE queue the way a z/hn slot (freed only
            # by the NEXT step's gate ACTs) would.
            hr2 = h[(t + 1) % 2][2]
            po = psum.tile([128, BL], F32, tag="in", bufs=2, name=f"po_{t}")
            for g in (0, 1):
                for j in (0, 1):
                    k = 2 * g + j
                    nc.tensor.matmul(po[32 * g:32 * g + 2, :],
                                     lhsT=wout[:, k, :], rhs=hr2[:, k, :],
                                     start=(j == 0), stop=(j == 1),
                                     tile_position=(0, 32 * g))
            return po

        def outproj_red(t, po):
            # cross-partition reduction, DVE-only (an ACT copy here would
            # block the queue ahead of layer 0's hot sigmoids): stage one
            # group to SBUF, add the other against it into obuf. The DVE
            # hits these ~po-stop, before its first layer-0 gate op.
            c1 = gates.tile([2, BL], F32, tag="os", name=f"c1_{t}")
            nc.vector.tensor_copy(c1[:], po[32:34, :])
            nc.vector.tensor_add(obuf[:, t, :], po[0:2, :], c1[:])

        def hmm_rz(pt, t, l, m, gate):
            """h-side r/z matmuls: fp8 DoubleRow, 256-contraction per pass.
            Keeps the PSUM group open for the x-side accumulation."""
            h8p = h8[t % 2][l]
            lo = gate * H + m * 128
            for kk in range(0, KT, 2):
                nc.tensor.matmul(pt[:], lhsT=whrz[:, l, kk:kk + 2, lo:lo + 128],
                                 rhs=h8p[:, kk:kk + 2, :], start=(kk == 0),
                                 stop=False, perf_mode=DRM)

        def hmm_n(pt, t, l, m):
            """h-side n matmul group; closes its PSUM group. Layer 1 runs
            fp8 DoubleRow off the existing h8 copy (numerics sim: 1.6e-2;
            l2 or any x-side n in fp8 blows the 2e-2 budget). Its weights
            are quantized UNSCALED so ph needs no descale in the STT."""
            lo = m * 128
            if l == 1 and NH8:
                h8p = h8[t % 2][l]
                for kk in range(0, KT, 2):
                    nc.tensor.matmul(pt[:], lhsT=whn8[:, kk:kk + 2, lo:lo + 128],
                                     rhs=h8p[:, kk:kk + 2, :], start=(kk == 0),
                                     stop=(kk == KT - 2), perf_mode=DRM)
                return
            hp = h[t % 2][l]
            for k in range(KT):
                nc.tensor.matmul(pt[:], lhsT=whn[:, l, k, lo:lo + 128],
                                 rhs=hp[:, k, :], start=(k == 0),
                                 stop=(k == KT - 1))

        def pend_block(t, l, x8in=None, xin=None):
            """Boundary fill-in: chunks 0+1's full h-side (n, r, z). 16
            matmuls that depend only on h(t-1), issued before anything
            that waits on the previous block's gate chain. pr1/pz1 land on
            the banks freed by the previous chunk-3's r/z ACTs, ~1us into
            the boundary — gap fillers by construction.

            At LAYER boundaries (x8in given = the current layer's h8/h
            output) the fill also takes the x-side pair-(0,1) DoubleRows
            and pin's first half: those read only chunks 0-1 of the
            previous layer's output, complete well before its chunk-3
            chain — the thing the boundary is waiting on."""
            ph0 = psum.tile([128, BL], F32, tag="hn", bufs=2,
                            name=f"ph_{t}_{l}_0")
            hmm_n(ph0, t, l, 0)
            pr0 = psum.tile([128, BL], F32, tag="r", name=f"pr_{t}_{l}_0")
            hmm_rz(pr0, t, l, 0, 0)
            pz0 = psum.tile([128, BL], F32, tag="z", name=f"pz_{t}_{l}_0")
            hmm_rz(pz0, t, l, 0, 1)
            pin0 = None
            if x8in is not None:
                nc.tensor.matmul(pr0[:], lhsT=wxrz[:, l, 0:2, 0:128],
                                 rhs=x8in[:, 0:2, :], start=False,
                                 stop=False, perf_mode=DRM)
                nc.tensor.matmul(pz0[:], lhsT=wxrz[:, l, 0:2, H:H + 128],
                                 rhs=x8in[:, 0:2, :], start=False,
                                 stop=False, perf_mode=DRM)
                # pin chunk0's first half; its "in" bank (the previous
                # layer's chunk-2 pin) freed a full chunk before the
                # boundary. The second half would land on the chunk-3 bank
                # — the late one — so it stays in-loop.
                pin0 = psum.tile([128, BL], F32, tag="in", bufs=2,
                                 name=f"pin_{t}_{l}_0")
                for k in (0, 1):
                    nc.tensor.matmul(pin0[:], lhsT=wxn[:, l, k, 0:128],
                                     rhs=xin[:, k, :], start=(k == 0),
                                     stop=False)
            ph1 = psum.tile([128, BL], F32, tag="hn", bufs=2,
                            name=f"ph_{t}_{l}_1")
            hmm_n(ph1, t, l, 1)
            pr1 = psum.tile([128, BL], F32, tag="r", name=f"pr_{t}_{l}_1")
            hmm_rz(pr1, t, l, 1, 0)
            pz1 = psum.tile([128, BL], F32, tag="z", name=f"pz_{t}_{l}_1")
            hmm_rz(pz1, t, l, 1, 1)
            if x8in is not None:
                nc.tensor.matmul(pr1[:], lhsT=wxrz[:, l, 0:2, 128:256],
                                 rhs=x8in[:, 0:2, :], start=False,
                                 stop=False, perf_mode=DRM)
                nc.tensor.matmul(pz1[:], lhsT=wxrz[:, l, 0:2, H + 128:H + 256],
                                 rhs=x8in[:, 0:2, :], start=False,
                                 stop=False, perf_mode=DRM)
            return ph0, pr0, pz0, ph1, pr1, pz1, pin0

        pend = None
        e1f, e2f = [], []
        for t in range(TPRED):
            hwr = h[(t + 1) % 2]
            h8w = h8[(t + 1) % 2]
            # input layer: x = relu(W_in @ h2 + b_in), on ACT so the px
            # PSUM bank recycling doesn't wait on the DVE gate chain. The
            # fp8 copy (layer-0 r,z input) is written first: it's on the
            # r-gate critical path.
            hr2 = h[t % 2][2]
            for m in range(MT):
                px = psum.tile([128, BL], F32, tag="in", bufs=2, name=f"px{m}")
                for k in range(KT):
                    nc.tensor.matmul(px[:],
                                     lhsT=win[:, k, m * 128:(m + 1) * 128],
                                     rhs=hr2[:, k, :],
                                     start=(k == 0), stop=(k == KT - 1))
                nc.scalar.activation(x8[:, m, :], px[:], AF.Relu,
                                     bias=bcol(48 + m))
                # bf16 copy on the DVE (reads PSUM + bias col, then max 0):
                # the ACT queue at step start must reach x8 m3 fast — the
                # whole of layer 0's x-side waits on it — while the DVE's
                # first gate op is ~5us out.
                nc.vector.tensor_scalar(x[:, m, :], px[:], bcol(48 + m),
                                        0.0, op0=ALU.add, op1=ALU.max)
            if t > 0:
                po_prev = outproj_mm(t - 1)
                outproj_red(t - 1, po_prev)
            if t == TPRED - 1:
                # overlap the bulk of the output writeback with the last
                # step (after outproj_red(10) has written obuf[:, 10])
                nc.sync.dma_start(out_d[:, :TPRED - 1, :], obuf[:, :TPRED - 1, :])
            for l in range(L):
                xin = x if l == 0 else hwr[l - 1]
                x8in = x8 if l == 0 else h8w[l - 1]
                hro = h[t % 2][l]
                tail_prev = None
                for m in range(MT):
                    lo = m * 128
                    hi = lo + 128
                    ph = pr = pz = pin = None
                    pend_x = False
                    if t > 0:
                        if m == 0:
                            ph, pr, pz = pend[0], pend[1], pend[2]
                            pend_x = pend[6] is not None
                            pin = pend[6]
                        elif m == 1:
                            ph, pr, pz = pend[3], pend[4], pend[5]
                            pend_x = pend[6] is not None
                        else:
                            ph = psum.tile([128, BL], F32, tag="hn", bufs=2,
                                           name=f"ph_{t}_{l}_{m}")
                            hmm_n(ph, t, l, m)
                        if pr is None:
                            pr = psum.tile([128, BL], F32, tag="r",
                                           name=f"pr_{t}_{l}_{m}")
                            hmm_rz(pr, t, l, m, 0)
                            pz = psum.tile([128, BL], F32, tag="z",
                                           name=f"pz_{t}_{l}_{m}")
                            hmm_rz(pz, t, l, m, 1)
                    else:
                        pr = psum.tile([128, BL], F32, tag="r",
                                       name=f"pr_{t}_{l}_{m}")
                        pz = psum.tile([128, BL], F32, tag="z",
                                       name=f"pz_{t}_{l}_{m}")
                    # x-side r,z: fp8 DoubleRow; closes the groups. Pair
                    # (0,1) for both gates ahead of pair (2,3): the pair
                    # only needs input chunks 0-1, giving the producer of
                    # chunk 3 (the previous layer's last gate chain) an
                    # extra matmul of slack (pend_x chunks had pair (0,1)
                    # pre-issued at the boundary). For layer 0's first
                    # chunks the input is the freshly relu'd x/x8, whose
                    # chunk-3 ACTs trail the PE — interleave pin's first
                    # half between the pairs so the PE never waits on the
                    # x8 m3 write.
                    if pin is None:
                        pin = psum.tile([128, BL], F32, tag="in", bufs=2,
                                        name=f"pin_{t}_{l}_{m}")
                        pin_ks = range(KT)
                    else:
                        pin_ks = (2, 3)
                    interleave = (l == 0 and m < 2)
                    kks = (2,) if (pend_x and m < 2) else (0, 2)
                    for kk in kks:
                        nc.tensor.matmul(pr[:],
                                         lhsT=wxrz[:, l, kk:kk + 2, lo:hi],
                                         rhs=x8in[:, kk:kk + 2, :],
                                         start=(t == 0 and kk == 0),
                                         stop=(kk == KT - 2), perf_mode=DRM)
                        nc.tensor.matmul(pz[:],
                                         lhsT=wxrz[:, l, kk:kk + 2, H + lo:H + hi],
                                         rhs=x8in[:, kk:kk + 2, :],
                                         start=(t == 0 and kk == 0),
                                         stop=(kk == KT - 2), perf_mode=DRM)
                        if interleave:
                            for k in (kk, kk + 1):
                                nc.tensor.matmul(pin[:],
                                                 lhsT=wxn[:, l, k, lo:hi],
                                                 rhs=xin[:, k, :],
                                                 start=(k == 0),
                                                 stop=(k == KT - 1))
                    if not interleave:
                        for k in pin_ks:
                            nc.tensor.matmul(pin[:],
                                             lhsT=wxn[:, l, k, lo:hi],
                                             rhs=xin[:, k, :], start=(k == 0),
                                             stop=(k == KT - 1))
                    if m == MT - 1:
                        # next block's boundary fill-in goes into the PE
                        # queue before the ops that wait on this chunk's
                        # gate chain. Only issued when the target step will
                        # actually consume it (t=0 has no h-side).
                        if l < L - 1:
                            if t > 0:
                                pend = pend_block(t, l + 1, h8w[l], hwr[l])
                        elif t < TPRED - 1:
                            pend = pend_block(t + 1, 0)

                    # gates for this feature chunk, in bf16 (2x DVE rate).
                    # h' = z*h + (1-z)*n computed as n + z*(h - n) — no
                    # (1-z) gate needed. The 1/16 ACT scale descales the
                    # x16-quantized fp8 r,z weights. For LAYER 2 ONLY the
                    # tanh-onward tail is deferred into the next chunk's
                    # section: every consumer of h2/h8[2] (px, outproj,
                    # next step's l2 h-side) needs chunk 3 last anyway, so
                    # the delay is free — and the in-order ACT queue then
                    # reaches chunk 3's r/z (which free the step-boundary
                    # pend's PSUM banks) without waiting on chunk 2's
                    # t2-dependent tanh. For layers 0/1 the tails stay in
                    # place: their h8 feeds the NEXT layer's x-side, where
                    # a one-chunk delay measurably stalls the PE.
                    final = (t == TPRED - 1 and l == L - 1)
                    defer = False
                    r = gates.tile([128, BL], CDT, tag="r", name=f"r{m}")
                    nc.scalar.activation(r[:], pr[:], AF.Sigmoid,
                                         bias=bcol(l * 16 + m), scale=SINV)
                    z = gates.tile([128, BL], CDT, tag="z", name=f"z{m}")
                    nc.scalar.activation(z[:], pz[:], AF.Sigmoid,
                                         bias=bcol(l * 16 + 4 + m), scale=SINV)
                    if final:
                        # final layer-2: h2(11) is only ever consumed by
                        # the output projection, so keep the n / z*(h-n)
                        # pieces and matmul them directly — no hnew install.
                        nt = gates.tile([128, BL], CDT, tag=f"e1f{m}",
                                        bufs=1, name=f"e1f{m}")
                        mzt = gates.tile([128, BL], CDT, tag=f"e2f{m}",
                                         bufs=1, name=f"e2f{m}")
                        e1f.append(nt)
                        e2f.append(mzt)
                    else:
                        nt = gates.tile([128, BL], CDT, tag="n", bufs=2,
                                        name=f"n{m}")
                        mzt = gates.tile([128, BL], CDT, tag="mz", bufs=2,
                                         name=f"mz{m}")
                    if final and m == MT - 1:
                        # last chunk of the last step: flush chunk 2's
                        # tail, then in-place half-batch pieces so the
                        # closing outproj's last dependency resolves early.
                        if tail_prev is not None:
                            tail_prev()
                            tail_prev = None
                        for sl in (slice(0, BL // 2), slice(BL // 2, BL)):
                            t1 = gates.tile([128, BL // 2], F32, tag="t1h",
                                            name=f"t1h_{sl.start}")
                            nc.vector.scalar_tensor_tensor(
                                t1[:], ph[:, sl], bcol(l * 16 + 8 + m),
                                r[:, sl], op0=ALU.add, op1=ALU.mult)
                            t2 = gates.tile([128, BL // 2], F32, tag="t1h",
                                            name=f"t2h_{sl.start}")
                            nc.vector.tensor_add(t2[:], t1[:], pin[:, sl])
                            nc.scalar.activation(nt[:, sl], t2[:], AF.Tanh,
                                                 bias=bcol(l * 16 + 12 + m))
                            d = gates.tile([128, BL // 2], CDT, tag="dh",
                                           bufs=1, name=f"dh{sl.start}")
                            nc.vector.tensor_sub(d[:], hro[:, m, sl],
                                                 nt[:, sl])
                            nc.vector.tensor_mul(mzt[:, sl], z[:, sl], d[:])
                        continue
                    # stage 1: the PSUM consumers (t1 reads ph, t2 reads pin)
                    t1 = gates.tile([128, BL], F32, tag="t1",
                                    name=f"t1_{m}")
                    if t > 0:
                        # t1 = (hn_psum + b_hh_n) * r
                        nc.vector.scalar_tensor_tensor(
                            t1[:], ph[:], bcol(l * 16 + 8 + m), r[:],
                            op0=ALU.add, op1=ALU.mult)
                    else:
                        nc.vector.tensor_scalar(t1[:], r[:],
                                                bcol(l * 16 + 8 + m),
                                                None, op0=ALU.mult)
                    t2 = gates.tile([128, BL], F32, tag="t1",
                                    name=f"t2_{m}")
                    nc.vector.tensor_add(t2[:], t1[:], pin[:])
                    if tail_prev is not None:
                        tail_prev()
                        tail_prev = None

                    def _tail(t=t, l=l, m=m, nt=nt, mzt=mzt, z=z, t2=t2,
                              hro=hro, hwr=hwr, h8w=h8w, final=final):
                        nc.scalar.activation(nt[:], t2[:], AF.Tanh,
                                             bias=bcol(l * 16 + 12 + m))
                        if t > 0:
                            d = gates.tile([128, BL], CDT, tag="d", bufs=1,
                                           name=f"d{m}")
                            nc.vector.tensor_sub(d[:], hro[:, m, :], nt[:])
                            nc.vector.tensor_mul(mzt[:], z[:], d[:])
                        else:
                            # h' = (1 - z) * n = n - z*n
                            nc.vector.tensor_mul(mzt[:], z[:], nt[:])
                        if final:
                            return
                        op = nc.vector.tensor_add if t > 0 else \
                            nc.vector.tensor_sub
                        gop = nc.gpsimd.tensor_add if t > 0 else \
                            nc.gpsimd.tensor_sub
                        if l == L - 1:
                            # h2 feeds the outproj + next input layer first.
                            # Its fp8 copy isn't consumed until the next
                            # step's layer-2 h-side, so the slow-but-idle
                            # GpSimd takes it off the DVE queue.
                            op(hwr[l][:, m, :], nt[:], mzt[:])
                            gop(h8w[l][:, m, :], nt[:], mzt[:])
                        else:
                            # h8 feeds the next layer's x-side DR matmuls
                            op(h8w[l][:, m, :], nt[:], mzt[:])
                            op(hwr[l][:, m, :], nt[:], mzt[:])
                    if defer:
                        tail_prev = _tail
                    else:
                        _tail()
                if tail_prev is not None:
                    tail_prev()
                    tail_prev = None
        # final outproj straight from the e1/e2 pieces, in two batch halves
        # so the first half's copy+DMA pipeline behind the second half's
        # matmuls; the very last matmul depends only on e2 chunk 3's second
        # half.
        for c in (0, BL // 2):
            sl = slice(c, c + BL // 2)
            poh = psum.tile([2, BL // 2], F32, tag="z", name=f"po_last{c}")
            for k in range(KT):
                nc.tensor.matmul(poh[:], lhsT=wout[:, k, :],
                                 rhs=e1f[k][:, sl], start=(k == 0), stop=False)
            for k in range(KT):
                nc.tensor.matmul(poh[:], lhsT=wout[:, k, :],
                                 rhs=e2f[k][:, sl], start=False,
                                 stop=(k == KT - 1))
            if c == 0:
                nc.scalar.copy(obuf[:, TPRED - 1, sl], poh[:])
                nc.sync.dma_start(out_d[:, TPRED - 1, sl],
                                  obuf[:, TPRED - 1, sl])
            else:
                nc.vector.tensor_scalar(obuf[:, TPRED - 1, sl], poh[:], 0.0,
                                        None, op0=ALU.add)
                nc.scalar.dma_start(out_d[:, TPRED - 1, sl],
                                    obuf[:, TPRED - 1, sl])

    nc.compile()
    return nc


def _to_dev(x):
    import ml_dtypes
    return np.ascontiguousarray(x, dtype=np.float32).astype(ml_dtypes.bfloat16)


def _to_fp8(x):
    import ml_dtypes
    y = np.clip(np.ascontiguousarray(x, dtype=np.float32) * SW, -240.0, 240.0)
    return y.astype(ml_dtypes.float8_e4m3)


def _prep_inputs(representation, W_in, b_in, W_ih, W_hh, b_ih, b_hh, W_out, b_out):
    rep_T = np.ascontiguousarray(representation.reshape(B, H).T)  # [H, B]
    # [H, X] -> [128, KT, X] partition-major tiling of the contraction dim
    win = _to_dev(W_in.T.reshape(KT, 128, H).transpose(1, 0, 2))
    wxrz = _to_fp8(np.transpose(W_ih[:, :2 * H], (0, 2, 1))
                   .reshape(L, KT, 128, 2 * H).transpose(2, 0, 1, 3))
    whrz = _to_fp8(np.transpose(W_hh[:, :2 * H], (0, 2, 1))
                   .reshape(L, KT, 128, 2 * H).transpose(2, 0, 1, 3))
    wxn = _to_dev(np.transpose(W_ih[:, 2 * H:], (0, 2, 1))
                  .reshape(L, KT, 128, H).transpose(2, 0, 1, 3))
    whn = _to_dev(np.transpose(W_hh[:, 2 * H:], (0, 2, 1))
                  .reshape(L, KT, 128, H).transpose(2, 0, 1, 3))
    import ml_dtypes
    whn8 = np.clip(np.ascontiguousarray(
        W_hh[1, 2 * H:].T.reshape(KT, 128, H).transpose(1, 0, 2),
        dtype=np.float32), -240.0, 240.0).astype(ml_dtypes.float8_e4m3)
    wout = _to_dev(W_out.T.reshape(KT, 128, 2).transpose(1, 0, 2))

    bias = np.zeros((128, NBIAS), dtype=np.float32)
    brz = (b_ih[:, :2 * H] + b_hh[:, :2 * H]).astype(np.float32)  # [L, 2H]
    for l in range(L):
        for g in range(2):
            for m in range(MT):
                bias[:, l * 16 + g * 4 + m] = brz[l, g * H + m * 128:
                                                  g * H + (m + 1) * 128]
        for m in range(MT):
            bias[:, l * 16 + 8 + m] = b_hh[l, 2 * H + m * 128:2 * H + (m + 1) * 128]
            bias[:, l * 16 + 12 + m] = b_ih[l, 2 * H + m * 128:2 * H + (m + 1) * 128]
            # negated z bias for q = sigmoid(-(pz/16 + b_z))
            bias[:, 53 + l * 4 + m] = -brz[l, H + m * 128:H + (m + 1) * 128]
    for m in range(MT):
        bias[:, 48 + m] = b_in[m * 128:(m + 1) * 128]

    shared = {"win": win, "wxrz": wxrz, "whrz": whrz, "wxn": wxn, "whn": whn,
              "whn8": whn8, "wout": wout, "bias": bias}
    in_maps = []
    for c in range(NCORES):
        m = dict(shared)
        rep_c = rep_T[:, c * BL:(c + 1) * BL]          # [H, BL]
        m["rep"] = _to_dev(rep_c.reshape(KT, 128, BL).transpose(1, 0, 2))
        in_maps.append(m)
    return in_maps


def _run(inputs, trace=False):
    if "nc" not in _CACHE:
        _CACHE["nc"] = _build()
    nc = _CACHE["nc"]
    in_maps = _prep_inputs(
        inputs["representation"], inputs["W_in"], inputs["b_in"],
        inputs["W_ih"], inputs["W_hh"], inputs["b_ih"], inputs["b_hh"],
        inputs["W_out"], inputs["b_out"])
    res = run_bass_kernel_spmd(nc, in_maps, core_ids=list(range(NCORES)),
                               trace=trace)
    # per-core out: [2, TPRED, BL] -> full [B, TPRED, 2]
    full = np.empty((B, TPRED, 2), dtype=np.float32)
    for c in range(NCORES):
        o = res.results[c]["out"]                      # [2, 12, BL]
        full[c * BL:(c + 1) * BL] = np.transpose(o, (2, 1, 0))
    full += inputs["b_out"].astype(np.float32)[None, None, :]
    return full, res


def kernel(**inputs) -> np.ndarray:
    out, _ = _run(inputs, trace=False)
    return out


def _setup_tracing():
    """Register the NTFF profile hook shim (test harness only)."""
    import types

    import trn_agent_boot.trn_boot as tb

    mod = types.ModuleType("antenv.axon_hooks")
    hook = [tb._ntff_profile_via_ctypes("/opt/axon/libaxon_pjrt.so")]
    mod.get_axon_ntff_profile_hook = lambda: hook[0]
    mod.set_axon_ntff_profile_hook = lambda h: hook.__setitem__(0, h)
    sys.modules["antenv.axon_hooks"] = mod
    import antenv
    antenv.axon_hooks = mod

    from concourse import bass_utils
    bass_utils.upload_artifacts = lambda tmpdir: str(tmpdir)
